# revision 2
# baseline (speedup 1.0000x reference)
"""Trainium2 Bass kernel for nn_Minerva_with_encoding (retrieval_knn).

Math (reference):
    pos_ids = argmin_j |R - enc_ids[j]|        [M]
    R_enc   = pos_encoding[pos_ids]            [M, 4]
    Xw = X @ Wx_w.T + Wx_b                     [N, 768]
    Dw = D @ Wd_w.T + Wd_b                     [M, 768]
    a  = Xw @ Dw.T                             [N, M]
    a  = sign(a) * |a|^2  ( = a * |a| )
    echo = a @ R_enc                           [N, 4]
    out  = echo @ We_w.T + We_b                [N, 1]

Strategy:
  * Fold We into the encoding gather on host: v = R_enc @ We_w.T  [M, 1],
    so out = a @ v + We_b.  The argmin/gather (1M flops) runs on host.
  * Shard the exemplar dim M=8192 across 8 cores (M/8 = 1024 each).
  * Each core computes, all in transposed [feature-on-partitions] layout:
      XwT [768, 4096] = WxT.T-tiles @ XT-tiles (+bias)    (replicated)
      DwT [768, 1024] = WdT.T-tiles @ DT-tiles (+bias)    (its shard)
      aT tiles [128m, 512n] = DwT-slices.T @ XwT-slices   (PSUM, fp32 acc)
      act = a * |a|  (ACT abs PSUM->SBUF, DVE mult PSUM x SBUF)
      partial[1, n] += v_m.T @ act                        (PE reduction)
  * Host sums the 8 partial [4096] vectors and adds We_b.
  * All matmul operands are float32r: full PE rate (1 cycle/row at N=512)
    with ~1e-4 max-rel accuracy.
"""

import numpy as np

import concourse.bacc as bacc
import concourse.mybir as mybir
import concourse.tile as tile
from concourse.bass_utils import run_bass_kernel_spmd

F32 = mybir.dt.float32
F32R = mybir.dt.float32r

N_CORES = 8
N_Q = 4096  # query rows
N_D = 8192  # exemplar rows (sharded)
D_IN = 768  # input features
REP = 768  # projection features
M_SH = N_D // N_CORES  # 1024 exemplars per core

DT_TILES = D_IN // 128  # 6
RT_TILES = REP // 128  # 6
NCH = 512  # n-chunk (moving free size)
N_CHUNKS = N_Q // NCH  # 8
ND_CHUNKS = M_SH // NCH  # 2 (for the D projection)
M_TILES = M_SH // 128  # 8

_CACHED = {}


def _build_nc():
    nc = bacc.Bacc(
        "TRN2", target_bir_lowering=False, debug=False, num_devices=N_CORES
    )
    xt = nc.declare_dram_parameter("xt", [D_IN, N_Q], F32R, isOutput=False)
    dtm = nc.declare_dram_parameter("dtm", [D_IN, M_SH], F32R, isOutput=False)
    wxt = nc.declare_dram_parameter("wxt", [D_IN, REP], F32R, isOutput=False)
    wdt = nc.declare_dram_parameter("wdt", [D_IN, REP], F32R, isOutput=False)
    bx = nc.declare_dram_parameter("bx", [128, RT_TILES], F32, isOutput=False)
    bd = nc.declare_dram_parameter("bd", [128, RT_TILES], F32, isOutput=False)
    vsh = nc.declare_dram_parameter("v", [128, M_TILES], F32R, isOutput=False)
    partial = nc.declare_dram_parameter("partial", [1, N_Q], F32, isOutput=True)

    with tile.TileContext(nc) as tc:
        with (
            tc.tile_pool(name="wxp", bufs=1) as wxp,
            tc.tile_pool(name="xwp", bufs=1) as xwp,
            tc.tile_pool(name="dwp", bufs=1) as dwp,
            tc.tile_pool(name="srcp", bufs=2) as srcp,
            tc.tile_pool(name="smallp", bufs=1) as smallp,
            tc.tile_pool(name="outp", bufs=2) as outp,
            tc.tile_pool(name="pp", bufs=2, space="PSUM") as pp,
            tc.tile_pool(name="redp", bufs=2, space="PSUM") as redp,
        ):
            # --- persistent small tiles -------------------------------------
            bx_sb = smallp.tile([128, RT_TILES], F32, tag="bx")
            nc.sync.dma_start(out=bx_sb, in_=bx[:, :])
            bd_sb = smallp.tile([128, RT_TILES], F32, tag="bd")
            nc.sync.dma_start(out=bd_sb, in_=bd[:, :])
            v_sb = smallp.tile([128, M_TILES], F32R, tag="v")
            nc.sync.dma_start(out=v_sb, in_=vsh[:, :])

            # --- weights (WxT persistent; WdT in a released-early pool) -----
            wx_sb = []
            for d in range(DT_TILES):
                wx_t = wxp.tile([128, REP], F32R, tag=f"wx{d}")
                nc.sync.dma_start(out=wx_t, in_=wxt[d * 128 : (d + 1) * 128, :])
                wx_sb.append(wx_t)

            # persistent projection outputs
            xw_sb = [
                xwp.tile([128, N_Q], F32R, tag=f"xw{r}", name=f"xw{r}")
                for r in range(RT_TILES)
            ]
            dw_sb = [
                dwp.tile([128, M_SH], F32R, tag=f"dw{r}", name=f"dw{r}")
                for r in range(RT_TILES)
            ]

            def project(src_dram, n_total, w_tiles, bias_sb, dst_tiles):
                """dst[r][:, chunk] = sum_d w[d][:,r-slice].T @ src[d][:,chunk] + b."""
                nchunks = n_total // NCH
                for n in range(nchunks):
                    src_sb = []
                    for d in range(DT_TILES):
                        s = srcp.tile([128, NCH], F32R, tag=f"src{d}", name=f"src{d}")
                        nc.sync.dma_start(
                            out=s,
                            in_=src_dram[
                                d * 128 : (d + 1) * 128, n * NCH : (n + 1) * NCH
                            ],
                        )
                        src_sb.append(s)
                    for r in range(RT_TILES):
                        ps = pp.tile([128, NCH], F32, tag="proj", name="proj_ps")
                        for d in range(DT_TILES):
                            nc.tensor.matmul(
                                ps,
                                w_tiles[d][:, r * 128 : (r + 1) * 128],
                                src_sb[d],
                                start=(d == 0),
                                stop=(d == DT_TILES - 1),
                            )
                        # psum -> sbuf with per-partition bias add (ACT engine)
                        nc.scalar.activation(
                            dst_tiles[r][:, n * NCH : (n + 1) * NCH],
                            ps,
                            mybir.ActivationFunctionType.Identity,
                            bias=bias_sb[:, r : r + 1],
                            scale=1.0,
                        )

            # --- phase A: projections ---------------------------------------
            with tc.tile_pool(name="wdp", bufs=1) as wdp:
                wd_sb = []
                for d in range(DT_TILES):
                    wd_t = wdp.tile([128, REP], F32R, tag=f"wd{d}")
                    nc.sync.dma_start(out=wd_t, in_=wdt[d * 128 : (d + 1) * 128, :])
                    wd_sb.append(wd_t)
                project(dtm, M_SH, wd_sb, bd_sb, dw_sb)
                project(xt, N_Q, wx_sb, bx_sb, xw_sb)

            # --- phase B: scores + power-sign act + reduction ---------------
            with tc.tile_pool(name="actp", bufs=2) as actp:
                for n in range(N_CHUNKS):
                    red_ps = redp.tile([1, NCH], F32, tag="red", name="red_ps")
                    pending = None  # (m, act_tile) awaiting its reduction MM
                    for m in range(M_TILES):
                        a_ps = pp.tile(
                            [128, NCH], F32, tag="a", bufs=3, name="a_ps"
                        )
                        for r in range(RT_TILES):
                            nc.tensor.matmul(
                                a_ps,
                                dw_sb[r][:, m * 128 : (m + 1) * 128],
                                xw_sb[r][:, n * NCH : (n + 1) * NCH],
                                start=(r == 0),
                                stop=(r == RT_TILES - 1),
                            )
                        # act = a * |a|
                        abs_t = actp.tile(
                            [128, NCH], F32R, tag="abs", name="abs_t"
                        )
                        nc.scalar.activation(
                            abs_t, a_ps, mybir.ActivationFunctionType.Abs
                        )
                        act_t = actp.tile(
                            [128, NCH], F32R, tag="act", bufs=3, name="act_t"
                        )
                        nc.vector.tensor_tensor(
                            act_t, in0=a_ps, in1=abs_t, op=mybir.AluOpType.mult
                        )
                        # one-step software pipeline: reduce m-1 while m computes
                        if pending is not None:
                            pm, pact = pending
                            nc.tensor.matmul(
                                red_ps,
                                v_sb[:, pm : pm + 1],
                                pact,
                                start=(pm == 0),
                                stop=False,
                            )
                        pending = (m, act_t)
                    pm, pact = pending
                    nc.tensor.matmul(
                        red_ps,
                        v_sb[:, pm : pm + 1],
                        pact,
                        start=False,
                        stop=True,
                    )
                    out_sb = outp.tile([1, NCH], F32, tag="out", name="out_sb")
                    nc.vector.tensor_copy(out_sb, red_ps)
                    nc.sync.dma_start(
                        out=partial[0:1, n * NCH : (n + 1) * NCH], in_=out_sb
                    )

    nc.compile()
    return nc


def _get_nc():
    if "nc" not in _CACHED:
        _CACHED["nc"] = _build_nc()
    return _CACHED["nc"]


def make_in_maps(inputs):
    X = np.asarray(inputs["X"], dtype=np.float32)
    D = np.asarray(inputs["D"], dtype=np.float32)
    R = np.asarray(inputs["R"], dtype=np.float32)

    # --- host: nearest-encoding lookup, fold We into v ----------------------
    pos_ids = np.argmin(
        np.abs(R - np.asarray(inputs["encoding_ids"], np.float32)[None, :]),
        axis=1,
    )
    R_enc = np.asarray(inputs["pos_encoding"], np.float32)[pos_ids]  # [M, R_DIM]
    v = (
        R_enc.astype(np.float64) @ np.asarray(inputs["We_w"], np.float64).T
    ).astype(np.float32)  # [M, 1]

    XT = np.ascontiguousarray(X.T)  # [768, 4096]
    DTm = np.ascontiguousarray(D.T)  # [768, 8192]
    WxT = np.ascontiguousarray(np.asarray(inputs["Wx_w"], np.float32).T)
    WdT = np.ascontiguousarray(np.asarray(inputs["Wd_w"], np.float32).T)
    bx = np.ascontiguousarray(
        np.asarray(inputs["Wx_b"], np.float32).reshape(RT_TILES, 128).T
    )  # [128, 6]
    bd = np.ascontiguousarray(
        np.asarray(inputs["Wd_b"], np.float32).reshape(RT_TILES, 128).T
    )

    in_maps = []
    for c in range(N_CORES):
        ms = slice(c * M_SH, (c + 1) * M_SH)
        in_maps.append(
            {
                "xt": XT,
                "dtm": np.ascontiguousarray(DTm[:, ms]),
                "wxt": WxT,
                "wdt": WdT,
                "bx": bx,
                "bd": bd,
                "v": np.ascontiguousarray(
                    v[ms, 0].reshape(M_TILES, 128).T
                ),  # [128, 8]
            }
        )
    return in_maps


def kernel(
    X, D, R, Wx_w, Wx_b, Wd_w, Wd_b, We_w, We_b, encoding_ids, pos_encoding
):
    in_maps = make_in_maps(
        {
            "X": X,
            "D": D,
            "R": R,
            "Wx_w": Wx_w,
            "Wx_b": Wx_b,
            "Wd_w": Wd_w,
            "Wd_b": Wd_b,
            "We_w": We_w,
            "We_b": We_b,
            "encoding_ids": encoding_ids,
            "pos_encoding": pos_encoding,
        }
    )
    We_b = np.asarray(We_b)

    nc = _get_nc()
    res = run_bass_kernel_spmd(nc, in_maps, list(range(N_CORES)))
    partials = np.stack(
        [np.asarray(res.results[c]["partial"])[0] for c in range(N_CORES)]
    )  # [8, 4096]
    out = partials.astype(np.float64).sum(axis=0)[:, None] + np.asarray(
        We_b, np.float64
    )[None, :]
    return out.astype(np.float32)


# revision 31
# speedup vs baseline: 1.8146x; 1.8146x over previous
"""Trainium2 Bass kernel for nn_Minerva_with_encoding (retrieval_knn).

Math (reference):
    pos_ids = argmin_j |R - enc_ids[j]|        [M]
    R_enc   = pos_encoding[pos_ids]            [M, 4]
    Xw = X @ Wx_w.T + Wx_b                     [N, 768]
    Dw = D @ Wd_w.T + Wd_b                     [M, 768]
    a  = Xw @ Dw.T                             [N, M]
    a  = sign(a) * |a|^2  ( = a * |a| )
    echo = a @ R_enc                           [N, 4]
    out  = echo @ We_w.T + We_b                [N, 1]

Strategy:
  * Host folds the two projections into one:  with A = Wx_w.T, B = Wd_w.T,
        a = X @ C @ D.T + p[n] + q[m] + c0
    where C = A @ B.T = Wx_w.T @ Wd_w   [768, 768]   (host, fp64)
          p = X @ (A @ Wd_b)  [N],  q = D @ (B @ Wx_b)  [M],  c0 = Wx_b.Wd_b.
    Raw D.T then streams straight into the score matmul — no on-device
    projection of D at all.
  * Host folds We into the encoding gather: v = R_enc @ We_w.T  [M, 1], so
    out = act(a) @ v + We_b.  argmin/gather (1M flops) runs on host.
  * Because D needs no projection, the optimal sharding is pure-N: each of
    the 8 cores takes a 512-query slab and the FULL exemplar set.  The
    per-core G projection (X-slab @ C) is 1/8 of the total G work — zero
    replicated compute.  Host output is a plain concat (+We_b).
  * Per core, transposed [feature-on-partitions] layout:
      GT [768, 512]  = C-tiles.T @ XT-tiles                (PE)
      aT tiles [128m, 512n] = DT-slices.T @ GT             (PE, PSUM fp32)
      s = a + q[m] + p[n]   (one DVE scalar_tensor_tensor pass)
      act = s * |s|         (ACT abs, DVE mult)
      partial[1, 512] += v_m.T @ act                       (PE reduction,
                                                            lag-3 pipelined)
    D.T (24 MB) streams through SBUF in [128, 6, 512] blocks — ONE DMA per
    block (DMA cost here is per-instruction as much as per-byte).
  * DMA choreography for the serial ~350 GB/s pipe: X slab, then C in per-r
    slices (~1.1 us each, pacing the ~1.3 us phase-A groups), the fused
    q/v/p vector early, then D.T chunk 0 in halves so phase B starts on the
    first half.  Throwaway warm-up matmuls lift the PE P-state during the
    initial DMA fill.
  * All matmul operands are float32r: full PE rate (1 cycle/row at 512-wide
    moving operand), ~1e-4 max-rel accuracy.
"""

import numpy as np

import concourse.bacc as bacc
import concourse.mybir as mybir
import concourse.tile as tile
from concourse.bass_utils import run_bass_kernel_spmd

F32 = mybir.dt.float32
F32R = mybir.dt.float32r

N_CORES = 8
N_Q = 4096  # query rows
N_D = 8192  # exemplar rows
D_IN = 768  # input features
REP = 768  # projection features

N_SL = N_Q // N_CORES  # 512-query slab per core
M_SL = N_D  # full exemplar set per core

DT_TILES = D_IN // 128  # 6
RT_TILES = REP // 128  # 6 (output dim of C)
NCH = 512  # moving-chunk size (= N_SL)
M_TILES = M_SL // 128  # 64
MC_TOTAL = M_SL // NCH  # 16 D.T m-chunks to stream
RED_LAG = 3  # reduction trails the score matmuls by this many m-tiles
WARMUP_MM = 4  # throwaway matmuls to warm the PE during the DMA fill

_CACHED = {}


def _build_nc():
    nc = bacc.Bacc(
        "TRN2", target_bir_lowering=False, debug=False, num_devices=N_CORES
    )
    xt = nc.declare_dram_parameter("xt", [D_IN, N_SL], F32R, isOutput=False)
    dtm = nc.declare_dram_parameter("dtm", [D_IN, M_SL], F32R, isOutput=False)
    cm = nc.declare_dram_parameter("cm", [D_IN, REP], F32R, isOutput=False)
    qvp = nc.declare_dram_parameter(
        "qvp", [128, 2 * M_TILES + N_SL], F32R, isOutput=False
    )
    partial = nc.declare_dram_parameter("partial", [1, N_SL], F32, isOutput=True)

    with tile.TileContext(nc) as tc:
        with (
            tc.tile_pool(name="cp", bufs=1) as cp,
            tc.tile_pool(name="gp", bufs=1) as gp,
            tc.tile_pool(name="srcp", bufs=1) as srcp,
            tc.tile_pool(name="dtp", bufs=4) as dtp,
            tc.tile_pool(name="smallp", bufs=1) as smallp,
            tc.tile_pool(name="actp", bufs=2) as actp,
            tc.tile_pool(name="pp", bufs=5, space="PSUM") as pp,
            tc.tile_pool(name="redp", bufs=1, space="PSUM") as redp,
        ):
            # PE warm-up: throwaway matmuls on scratch SBUF run inside the
            # initial DMA fill and lift the PE out of its cold P-state.
            warm_sb = smallp.tile([128, NCH], F32, tag="warm")
            nc.vector.memset(warm_sb, 0.0)
            warm_ps = pp.tile([128, NCH], F32, tag="warm", bufs=1, name="warm_ps")
            for _ in range(WARMUP_MM):
                nc.tensor.matmul(
                    warm_ps, warm_sb[:, 0:128], warm_sb, start=True, stop=True
                )

            # 3D [partition, d-tile, col] views of the [768, *] DRAM operands:
            # one DMA moves a whole multi-tile block.
            xt3 = xt[:, :].rearrange("(t p) m -> p t m", p=128)
            cm3 = cm[:, :].rearrange("(t p) m -> p t m", p=128)
            dtm3 = dtm[:, :].rearrange("(t p) m -> p t m", p=128)

            src_all = srcp.tile([128, DT_TILES, NCH], F32R, tag="src")
            nc.sync.dma_start(out=src_all, in_=xt3)
            c_all = cp.tile([128, DT_TILES, REP], F32R, tag="c")
            nc.sync.dma_start(out=c_all[:, :, 0:128], in_=cm3[:, :, 0:128])
            qvp_sb = smallp.tile([128, 2 * M_TILES + N_SL], F32R, tag="qvp")
            nc.sync.dma_start(out=qvp_sb, in_=qvp[:, :])
            for r in range(1, RT_TILES):
                nc.sync.dma_start(
                    out=c_all[:, :, r * 128 : (r + 1) * 128],
                    in_=cm3[:, :, r * 128 : (r + 1) * 128],
                )
            qb_sb = qvp_sb[:, 0:M_TILES]
            v_sb = qvp_sb[:, M_TILES : 2 * M_TILES]
            p_sb = qvp_sb[:, 2 * M_TILES :]

            # D.T streaming chunks, one DMA per [128, 6, 512] block
            dt_tiles = {}

            def load_dt_mchunk(mc, halves=False):
                t = dtp.tile([128, DT_TILES, NCH], F32R, tag="dt", name="dt")
                if halves:
                    nc.sync.dma_start(
                        out=t[:, :, 0 : NCH // 2],
                        in_=dtm3[:, :, mc * NCH : mc * NCH + NCH // 2],
                    )
                    nc.sync.dma_start(
                        out=t[:, :, NCH // 2 : NCH],
                        in_=dtm3[:, :, mc * NCH + NCH // 2 : (mc + 1) * NCH],
                    )
                else:
                    nc.sync.dma_start(
                        out=t, in_=dtm3[:, :, mc * NCH : (mc + 1) * NCH]
                    )
                dt_tiles[mc] = t

            load_dt_mchunk(0, halves=True)

            g_sb = [
                gp.tile([128, N_SL], F32R, tag=f"g{r}", name=f"g{r}")
                for r in range(RT_TILES)
            ]

            # --- phase A: GT = C.T-tiles @ XT-tiles (6 groups, ~8 us) --------
            # G copies alternate DVE/ACT so the last two drain in parallel.
            for r in range(RT_TILES):
                ps = pp.tile([128, NCH], F32, tag="big", name="proj_ps")
                for d in range(DT_TILES):
                    nc.tensor.matmul(
                        ps,
                        c_all[:, d, r * 128 : (r + 1) * 128],
                        src_all[:, d, :],
                        start=(d == 0),
                        stop=(d == DT_TILES - 1),
                    )
                if r % 2 == 0:
                    nc.vector.tensor_copy(g_sb[r], ps)
                else:
                    nc.scalar.copy(g_sb[r], ps)

            # --- phase B: scores + corrections + power-sign + reduction -----
            mc_loaded = 1
            red_ps = redp.tile([1, NCH], F32, tag="red", name="red_ps")
            pending = []  # (m, act_tile) awaiting their reduction MM
            for m in range(M_TILES):
                mc, off = divmod(m * 128, NCH)
                want = min(MC_TOTAL, mc + 3)
                while mc_loaded < want:
                    load_dt_mchunk(mc_loaded)
                    mc_loaded += 1
                a_ps = pp.tile([128, NCH], F32, tag="big", name="a_ps")
                for r in range(RT_TILES):
                    nc.tensor.matmul(
                        a_ps,
                        dt_tiles[mc][:, r, off : off + 128],
                        g_sb[r],
                        start=(r == 0),
                        stop=(r == RT_TILES - 1),
                    )
                # s = a + q[m] + p[n]  (single DVE pass, psum -> sbuf)
                s_t = actp.tile([128, NCH], F32R, tag="s", bufs=3, name="s_t")
                nc.vector.scalar_tensor_tensor(
                    s_t,
                    in0=a_ps,
                    scalar=qb_sb[:, m : m + 1],
                    in1=p_sb,
                    op0=mybir.AluOpType.add,
                    op1=mybir.AluOpType.add,
                )
                # act = s * |s|
                abs_t = actp.tile([128, NCH], F32R, tag="abs", bufs=3, name="abs_t")
                nc.scalar.activation(abs_t, s_t, mybir.ActivationFunctionType.Abs)
                act_t = actp.tile(
                    [128, NCH], F32R, tag="act", bufs=RED_LAG + 2, name="act_t"
                )
                nc.vector.tensor_tensor(
                    act_t, in0=s_t, in1=abs_t, op=mybir.AluOpType.mult
                )
                pending.append((m, act_t))
                # software pipeline: reduce m-RED_LAG while m computes
                if len(pending) > RED_LAG:
                    pm, pact = pending.pop(0)
                    nc.tensor.matmul(
                        red_ps,
                        v_sb[:, pm : pm + 1],
                        pact,
                        start=(pm == 0),
                        stop=False,
                    )
            for pm, pact in pending:
                nc.tensor.matmul(
                    red_ps,
                    v_sb[:, pm : pm + 1],
                    pact,
                    start=(pm == 0),
                    stop=(pm == M_TILES - 1),
                )
            out_sb = actp.tile([1, NCH], F32, tag="out", bufs=1, name="out_sb")
            nc.scalar.copy(out_sb, red_ps)
            nc.sync.dma_start(out=partial[0:1, :], in_=out_sb)

    nc.compile()
    return nc


def _get_nc():
    if "nc" not in _CACHED:
        _CACHED["nc"] = _build_nc()
    return _CACHED["nc"]


def make_in_maps(inputs):
    X = np.asarray(inputs["X"], dtype=np.float32)
    D = np.asarray(inputs["D"], dtype=np.float32)
    R = np.asarray(inputs["R"], dtype=np.float32)
    Wx_w = np.asarray(inputs["Wx_w"], np.float32)
    Wd_w = np.asarray(inputs["Wd_w"], np.float32)
    Wx_b = np.asarray(inputs["Wx_b"], np.float32)
    Wd_b = np.asarray(inputs["Wd_b"], np.float32)

    # --- host: nearest-encoding lookup, fold We into v ----------------------
    pos_ids = np.argmin(
        np.abs(R - np.asarray(inputs["encoding_ids"], np.float32)[None, :]),
        axis=1,
    )
    R_enc = np.asarray(inputs["pos_encoding"], np.float32)[pos_ids]  # [M, R_DIM]
    v = (
        R_enc.astype(np.float64) @ np.asarray(inputs["We_w"], np.float64).T
    ).astype(np.float32)  # [M, 1]

    # --- host: fold the two projections (fp64) ------------------------------
    A64 = Wx_w.T.astype(np.float64)  # [d, r]
    B64 = Wd_w.T.astype(np.float64)  # [d', r]
    C = np.ascontiguousarray((A64 @ B64.T).astype(np.float32))  # [d, d']
    p = (X.astype(np.float64) @ (A64 @ Wd_b.astype(np.float64))).astype(
        np.float32
    )  # [N]
    q = D.astype(np.float64) @ (B64 @ Wx_b.astype(np.float64))  # [M] f64
    c0 = float(Wx_b.astype(np.float64) @ Wd_b.astype(np.float64))
    qc = (q + c0).astype(np.float32)  # [M]

    XT = np.ascontiguousarray(X.T)  # [768, 4096]
    DTm = np.ascontiguousarray(D.T)  # [768, 8192]
    qbm = np.ascontiguousarray(qc.reshape(M_TILES, 128).T)  # [128, 64]
    vb = np.ascontiguousarray(v[:, 0].reshape(M_TILES, 128).T)  # [128, 64]

    in_maps = []
    for c in range(N_CORES):
        nsl = slice(c * N_SL, (c + 1) * N_SL)
        qvp = np.concatenate(
            [qbm, vb, np.broadcast_to(p[nsl][None, :], (128, N_SL))], axis=1
        )
        in_maps.append(
            {
                "xt": np.ascontiguousarray(XT[:, nsl]),
                "dtm": DTm,
                "cm": C,
                "qvp": np.ascontiguousarray(qvp),
            }
        )
    return in_maps


def gather_output(results, We_b):
    """results: list of per-core dicts with 'partial' [1, N_SL]."""
    out = np.concatenate(
        [np.asarray(results[c]["partial"])[0] for c in range(N_CORES)]
    ).astype(np.float64)[:, None]
    out += np.asarray(We_b, np.float64)[None, :]
    return out.astype(np.float32)


def kernel(
    X, D, R, Wx_w, Wx_b, Wd_w, Wd_b, We_w, We_b, encoding_ids, pos_encoding
):
    in_maps = make_in_maps(
        {
            "X": X,
            "D": D,
            "R": R,
            "Wx_w": Wx_w,
            "Wx_b": Wx_b,
            "Wd_w": Wd_w,
            "Wd_b": Wd_b,
            "We_w": We_w,
            "We_b": We_b,
            "encoding_ids": encoding_ids,
            "pos_encoding": pos_encoding,
        }
    )
    nc = _get_nc()
    res = run_bass_kernel_spmd(nc, in_maps, list(range(N_CORES)))
    return gather_output(res.results, We_b)
